# revision 34
# baseline (speedup 1.0000x reference)
"""MLA (multi-head latent attention) Trainium2 kernel.

Sharding: 8 cores = 2 (batch) x 4 (head groups of 4 heads).
Each core computes, for its batch b and heads [4g, 4g+4):
  latents kv_d/q_d (replicated within the batch group), per-head
  up-projections + RoPE, causal SDPA, and a partial o_proj
  out_core[o, q] = sum_{d in core's 512 head-dims} W_o[o, d] * y[d, q].
Host sums the 4 partials per batch (the all-reduce step of the hint,
performed at unshard time) and transposes to [S, H].

All matmuls run in bf16 with fp32 PSUM accumulation.

Phase 1 streams x once: per 512-col quarter, 6 matmuls per x k-tile
produce kv_d, q_d and raw rope-k together.  SDPA row-sums are
pre-reduced on DVE (bf16) with a single ones-matmul per (head,
q-block) for the partition reduction; softmax tails and o_proj are
deferred in program order so the Tensor engine never waits on them.
"""

import numpy as np
import ml_dtypes

import concourse.bass as bass
import concourse.mybir as mybir
import concourse.tile as tile
from concourse import bacc
from concourse._compat import get_trn_type
from concourse.bass_utils import run_bass_kernel_spmd

H = 2048
NH = 16
HD = 128           # head dim
RD = 64            # rotary dim
RH = 32            # rotary half
LAT = 256
B = 2
S = 2048
BASE = 10000.0
N_CORES = 8
HEADS_PER_CORE = 4
P = 128
NQB = S // 512     # 4 query blocks of 512
NKI = S // 128     # 16 key chunks of 128
SCALE = 1.0 / np.sqrt(float(HD))
EXP_BIAS = -4.0

BF16 = mybir.dt.bfloat16
F32 = mybir.dt.float32
_bf = ml_dtypes.bfloat16


def _mm(nc, out, lhsT, rhs, start, stop):
    nc.tensor.matmul(out, lhsT, rhs, start=start, stop=stop)


def build_program(nrep=1, bench_io=False):
    nc = bacc.Bacc(
        get_trn_type() or "TRN2",
        target_bir_lowering=False,
        debug=False,
        num_devices=N_CORES,
    )

    if bench_io:
        dummy = nc.declare_dram_parameter("bdummy", [1, 128], F32, isOutput=False)
        xT = nc.dram_tensor("xT", [H, S], BF16)
        w_kvd = nc.dram_tensor("w_kvd", [P, 16, LAT], BF16)
        w_qd = nc.dram_tensor("w_qd", [P, 16, LAT], BF16)
        w_rk = nc.dram_tensor("w_rk", [P, 16, 256], BF16)
        w_qc = nc.dram_tensor("w_qc", [P, 2, 512], BF16)
        w_kc = nc.dram_tensor("w_kc", [P, 2, 256], BF16)
        w_v = nc.dram_tensor("w_v", [P, 2, 512], BF16)
        w_o = nc.dram_tensor("w_o", [P, 4, H], BF16)
        cosA = nc.dram_tensor("cosA", [P, S], BF16)
        sinB = nc.dram_tensor("sinB", [P, S], BF16)
        masks = nc.dram_tensor("masks", [P, 4, 512], BF16)
        ident = nc.dram_tensor("ident", [P, P], BF16)
        out = nc.dram_tensor("outs", [H, S], BF16)
        outp = nc.declare_dram_parameter("out", [1, 128], BF16, isOutput=True)
    else:
        xT = nc.declare_dram_parameter("xT", [H, S], BF16, isOutput=False)
        w_kvd = nc.declare_dram_parameter("w_kvd", [P, 16, LAT], BF16, isOutput=False)
        w_qd = nc.declare_dram_parameter("w_qd", [P, 16, LAT], BF16, isOutput=False)
        w_rk = nc.declare_dram_parameter("w_rk", [P, 16, 256], BF16, isOutput=False)
        w_qc = nc.declare_dram_parameter("w_qc", [P, 2, 512], BF16, isOutput=False)
        w_kc = nc.declare_dram_parameter("w_kc", [P, 2, 256], BF16, isOutput=False)
        w_v = nc.declare_dram_parameter("w_v", [P, 2, 512], BF16, isOutput=False)
        w_o = nc.declare_dram_parameter("w_o", [P, 4, H], BF16, isOutput=False)
        cosA = nc.declare_dram_parameter("cosA", [P, S], BF16, isOutput=False)
        sinB = nc.declare_dram_parameter("sinB", [P, S], BF16, isOutput=False)
        masks = nc.declare_dram_parameter("masks", [P, 4, 512], BF16, isOutput=False)
        ident = nc.declare_dram_parameter("ident", [P, P], BF16, isOutput=False)
        out = nc.declare_dram_parameter("out", [H, S], BF16, isOutput=True)

    Exp = mybir.ActivationFunctionType.Exp

    scratch = (nc.dram_tensor("scratch", [H, S], BF16) if nrep > 1 else None)

    with tile.TileContext(nc) as tc:
      for rep in range(nrep):
        out_r = out if rep == nrep - 1 else scratch
        with (
            tc.tile_pool(name=f"wpool{rep}", bufs=1) as wpool,
            tc.tile_pool(name=f"main1_{rep}", bufs=1) as main1,
            tc.tile_pool(name=f"main2_{rep}", bufs=1) as main2,
            tc.tile_pool(name=f"ph1_{rep}", bufs=1) as ph1,
            tc.tile_pool(name=f"rot{rep}", bufs=3) as rot,
        ):
            # -------- persistent small tensors --------
            cos_sb = wpool.tile([P, S], BF16, tag="cos", name="cos")
            sin_sb = wpool.tile([P, S], BF16, tag="sin", name="sin")
            mask_sb = wpool.tile([P, 4, 512], BF16, tag="mask", name="mask")
            id_sb = wpool.tile([P, P], BF16, tag="ident", name="ident")
            ones_sb = wpool.tile([P, P], BF16, tag="ones", name="ones")
            nc.gpsimd.memset(ones_sb[:], 1.0)
            ebias_sb = wpool.tile([P, 1], F32, tag="ebias", name="ebias")
            nc.gpsimd.memset(ebias_sb[:], EXP_BIAS)
            wo_sb = wpool.tile([P, 4, H], BF16, tag="wo", name="wo")
            wqc_sb = wpool.tile([P, 2, 512], BF16, tag="wqc", name="wqc")
            wkc_sb = wpool.tile([P, 2, 256], BF16, tag="wkc", name="wkc")
            wv_sb = wpool.tile([P, 2, 512], BF16, tag="wv", name="wv")

            # -------- phase-1 outputs (latents + raw rope-k), bf16 --------
            kvd_sb = [main1.tile([P, S], BF16, tag=f"kvd{m}", name=f"kvd{m}")
                      for m in range(2)]
            qd_sb = [main1.tile([P, S], BF16, tag=f"qd{m}", name=f"qd{m}")
                     for m in range(2)]
            kraw = [main1.tile([P, S], BF16, tag=f"kraw{p}", name=f"kraw{p}")
                    for p in range(2)]

            # phase-1 weights: first k-chunks land before the x stream,
            # the bulk is split so arrival outpaces the k loop
            wkvd_sb = ph1.tile([P, 16, LAT], BF16, tag="wkvd", name="wkvd")
            wqd_sb = ph1.tile([P, 16, LAT], BF16, tag="wqd", name="wqd")
            wrk_sb = ph1.tile([P, 16, 256], BF16, tag="wrk", name="wrk")
            nc.sync.dma_start(wkvd_sb[:, 0:2, :], w_kvd[:, 0:2, :])
            nc.sync.dma_start(wqd_sb[:, 0:2, :], w_qd[:, 0:2, :])
            nc.sync.dma_start(wrk_sb[:, 0:2, :], w_rk[:, 0:2, :])

            # -------- per-head q/k tiles (dims on partitions), v, y --------
            # even head h: rows [0:64] content, [64:128] rope
            # odd  head h: rows [0:64] rope,    [64:128] content
            qT = [main2.tile([P, S], BF16, tag=f"qT{h}", name=f"qT{h}")
                  for h in range(4)]
            kT = [main2.tile([P, S], BF16, tag=f"kT{h}", name=f"kT{h}")
                  for h in range(4)]
            # v2[:, ki, p*256 + hv : +128] = v for head pair p, head h%2
            v2_sb = main2.tile([P, NKI, 512], BF16, tag="v2", name="v2")
            y_sb = [main2.tile([P, S], BF16, tag=f"y{h}", name=f"y{h}")
                    for h in range(4)]

            # -------- phases 1+2 fused: per 512-col quarter, the big
            # x-stream k-loop produces latents + raw rope-k; the
            # quarter's up-projections (k-content, q, v) run while the
            # NEXT quarter's k-loop keeps PE busy --------
            with tc.tile_pool(name=f"ps1_{rep}", bufs=1, space="PSUM") as ps1:

                def quarter_kloop(n4):
                    ns = slice(n4 * 512, (n4 + 1) * 512)
                    pkv = [ps1.tile([P, 512], F32, tag=f"pkv{m}",
                                    name=f"pkv{m}") for m in range(2)]
                    pqd = [ps1.tile([P, 512], F32, tag=f"pqd{m}",
                                    name=f"pqd{m}") for m in range(2)]
                    pkb = [ps1.tile([P, 512], F32, tag=f"pkb{p}",
                                    name=f"pkb{p}") for p in range(2)]
                    for k in range(16):
                        xtk = ph1.tile([P, 512], BF16, tag="xtk", name="xtk",
                                       bufs=8)
                        nc.sync.dma_start(
                            xtk[:],
                            xT[k * 128:(k + 1) * 128, ns],
                        )
                        if n4 == 0 and k in (1, 3, 5, 7, 9, 11, 13):
                            # bulk weights, arrival-paced 2-chunk pieces
                            ws = slice(k + 1, k + 3)
                            nc.sync.dma_start(wkvd_sb[:, ws, :],
                                              w_kvd[:, ws, :])
                            nc.sync.dma_start(wqd_sb[:, ws, :],
                                              w_qd[:, ws, :])
                            nc.sync.dma_start(wrk_sb[:, ws, :],
                                              w_rk[:, ws, :])
                        if n4 == 1 and k == 9:
                            nc.sync.dma_start(cos_sb[:], cosA[:, :])
                            nc.sync.dma_start(sin_sb[:], sinB[:, :])
                        if n4 == 1 and k == 11:
                            nc.sync.dma_start(wkc_sb[:], w_kc[:, :, :])
                            nc.sync.dma_start(wqc_sb[:], w_qc[:, :, :])
                        if n4 == 1 and k == 13:
                            nc.sync.dma_start(wv_sb[:], w_v[:, :, :])
                        if n4 == 2 and k == 2:
                            nc.sync.dma_start(id_sb[:], ident[:, :])
                            nc.sync.dma_start(mask_sb[:], masks[:, :, :])
                        if n4 == 2 and k == 8:
                            nc.sync.dma_start(wo_sb[:], w_o[:, :, :])
                        for m in range(2):
                            _mm(nc, pkv[m][:],
                                wkvd_sb[:, k, m * 128:(m + 1) * 128],
                                xtk[:], k == 0, k == 15)
                        for m in range(2):
                            _mm(nc, pqd[m][:],
                                wqd_sb[:, k, m * 128:(m + 1) * 128],
                                xtk[:], k == 0, k == 15)
                        for p in range(2):
                            _mm(nc, pkb[p][:],
                                wrk_sb[:, k, p * 128:(p + 1) * 128],
                                xtk[:], k == 0, k == 15)
                    return pkv, pqd, pkb

                def quarter_copies(n4, pkv, pqd, pkb):
                    ns = slice(n4 * 512, (n4 + 1) * 512)
                    # copies split across ACT/DVE
                    nc.scalar.copy(kvd_sb[0][:, ns], pkv[0][:])
                    nc.vector.tensor_copy(kvd_sb[1][:, ns], pkv[1][:])
                    nc.scalar.copy(qd_sb[0][:, ns], pqd[0][:])
                    nc.vector.tensor_copy(qd_sb[1][:, ns], pqd[1][:])
                    nc.scalar.copy(kraw[0][:, ns], pkb[0][:])
                    nc.vector.tensor_copy(kraw[1][:, ns], pkb[1][:])

                def quarter_krope(n4):
                    ns = slice(n4 * 512, (n4 + 1) * 512)
                    # rope on k for this quarter:
                    # kraw[p] rows [0:64]=h1 rope, [64:128]=h0 rope
                    for p in range(2):
                        h0, h1 = 2 * p, 2 * p + 1
                        swp = rot.tile([P, 512], BF16, tag="swp", name="swp",
                                       bufs=2)
                        nc.sync.dma_start(swp[0:32, :], kraw[p][32:64, ns])
                        nc.sync.dma_start(swp[32:64, :], kraw[p][0:32, ns])
                        nc.sync.dma_start(swp[64:96, :], kraw[p][96:128, ns])
                        nc.sync.dma_start(swp[96:128, :], kraw[p][64:96, ns])
                        nc.vector.tensor_mul(swp[:], swp[:], sin_sb[:, ns])
                        nc.vector.tensor_mul(
                            kT[h1][0:64, ns], kraw[p][0:64, ns],
                            cos_sb[0:64, ns]
                        )
                        nc.vector.tensor_add(
                            kT[h1][0:64, ns], kT[h1][0:64, ns], swp[0:64, :]
                        )
                        nc.vector.tensor_mul(
                            kT[h0][64:128, ns], kraw[p][64:128, ns],
                            cos_sb[64:128, ns]
                        )
                        nc.vector.tensor_add(
                            kT[h0][64:128, ns], kT[h0][64:128, ns],
                            swp[64:128, :]
                        )

                def quarter_up(n4, pool=None, ptag="ph2"):
                    # up-projections for quarter n4 (latents are ready)
                    pool = pool or ps1
                    ns = slice(n4 * 512, (n4 + 1) * 512)
                    # k content per pair: psum rows [0:64]=h0, [64:128]=h1
                    for p in range(2):
                        h0, h1 = 2 * p, 2 * p + 1
                        pt = pool.tile([P, 512], F32, tag=ptag, name="kcont",
                                       bufs=2)
                        for kc in range(2):
                            _mm(nc, pt[:],
                                wkc_sb[:, kc, p * 128:(p + 1) * 128],
                                kvd_sb[kc][:, ns], kc == 0, kc == 1)
                        nc.scalar.copy(kT[h0][0:64, ns], pt[0:64, :])
                        nc.vector.tensor_copy(kT[h1][64:128, ns],
                                              pt[64:128, :])
                    # q up-proj + rope per head
                    for h in range(4):
                        pt = pool.tile([P, 512], F32, tag=ptag, name="qcomb",
                                       bufs=2)
                        for kc in range(2):
                            _mm(nc, pt[:],
                                wqc_sb[:, kc, h * 128:(h + 1) * 128],
                                qd_sb[kc][:, ns], kc == 0, kc == 1)
                        nc.scalar.copy(qT[h][:, ns], pt[:])
                        r = 64 if h % 2 == 0 else 0
                        rs_ = slice(n4 * 512, (n4 + 1) * 512)
                        swp = rot.tile([P, 512], BF16, tag="swp", name="swpq",
                                       bufs=2)
                        nc.sync.dma_start(swp[r:r + 32, :],
                                          qT[h][r + 32:r + 64, rs_])
                        nc.sync.dma_start(swp[r + 32:r + 64, :],
                                          qT[h][r:r + 32, rs_])
                        nc.vector.tensor_mul(
                            qT[h][r:r + 64, ns], qT[h][r:r + 64, ns],
                            cos_sb[r:r + 64, ns]
                        )
                        nc.vector.tensor_mul(
                            swp[r:r + 64, :], swp[r:r + 64, :],
                            sin_sb[r:r + 64, ns]
                        )
                        nc.vector.tensor_add(
                            qT[h][r:r + 64, ns], qT[h][r:r + 64, ns],
                            swp[r:r + 64, :]
                        )
                    # v for this quarter's key chunks (both head pairs)
                    for i, s16 in enumerate(range(4 * n4, 4 * n4 + 4)):
                        pt = pool.tile([P, 512], F32, tag=ptag, name="vps",
                                       bufs=2)
                        for kc in range(2):
                            _mm(nc, pt[:],
                                kvd_sb[kc][:, s16 * 128:(s16 + 1) * 128],
                                wv_sb[:, kc, :], kc == 0, kc == 1)
                        if i % 2 == 0:
                            nc.vector.tensor_copy(v2_sb[:, s16, :], pt[:])
                        else:
                            nc.scalar.copy(v2_sb[:, s16, :], pt[:])

                # krope/up lag one quarter: they run while quarter n4+1's
                # k-loop owns the DMA stream, so their weights (issued in
                # quarter 1) never starve quarter 0's x tiles
                state = quarter_kloop(0)
                quarter_copies(0, *state)
                for n4 in range(1, 4):
                    state = quarter_kloop(n4)
                    quarter_krope(n4 - 1)
                    quarter_up(n4 - 1)
                    quarter_copies(n4, *state)
                quarter_krope(3)
                # quarter_up(3) is emitted inside the SDPA section after
                # the first head (which only needs quarter-0 products),
                # hiding its PSUM-copy chain under SDPA matmuls

            # -------- SDPA + o_proj --------
            # Emission schedule: per (h, qb) "core" = scores/exp/mask/pv +
            # DVE row-sum adds; "tail" = ones-matmul partition reduce +
            # rcp + y normalize, delayed one core-slot; o_proj(qb)
            # delayed past tail(3, qb) so PE never waits on DVE chains.
            with tc.tile_pool(name=f"ps3_{rep}", bufs=1, space="PSUM") as ps3:
                yps_t = {}
                racc_t = {}
                rsum_t = {}

                def sdpa_core(h, qb):
                    p = h // 2
                    hv = (h % 2) * 128
                    qs = slice(qb * 512, (qb + 1) * 512)
                    ngr = 2 * (qb + 1)    # groups of 2 k-chunks
                    # diagonal groups first so their extra mask step is
                    # off the tail
                    gs = [2 * qb, 2 * qb + 1] + list(range(2 * qb))
                    yps = ps3.tile([P, 512], F32, tag="ypv", name="ypv",
                                   bufs=2)
                    yps_t[(h, qb)] = yps
                    racc = rot.tile([P, 512], BF16, tag="racc", name="racc",
                                    bufs=2)
                    racc_t[(h, qb)] = racc
                    for gi, g in enumerate(gs):
                        diag = g // 2 == qb
                        scps = ps3.tile([P, 2, 512], F32, tag="sc",
                                        name="sc", bufs=2)
                        for j in range(2):
                            ki = 2 * g + j
                            _mm(nc, scps[:, j, :],
                                kT[h][:, ki * 128:(ki + 1) * 128],
                                qT[h][:, qs], True, not diag)
                            if diag:
                                # additive causal mask folded into PSUM
                                _mm(nc, scps[:, j, :], id_sb[:],
                                    mask_sb[:, 2 * (g % 2) + j, :],
                                    False, True)
                        prb = rot.tile([P, 2, 512], BF16, tag="prb",
                                       name="prb")
                        nc.scalar.activation(
                            prb[:], scps[:], Exp, bias=ebias_sb[:],
                            scale=SCALE
                        )
                        for j in range(2):
                            _mm(nc, yps[:],
                                v2_sb[:, 2 * g + j, p * 256 + hv:
                                      p * 256 + hv + 128],
                                prb[:, j, :], gi == 0 and j == 0,
                                gi == ngr - 1 and j == 1)
                        # row-sum pre-reduction on DVE (bf16, 2x rate)
                        if gi == 0:
                            nc.vector.tensor_add(racc[:], prb[:, 0, :],
                                                 prb[:, 1, :])
                        else:
                            tmp = rot.tile([P, 512], BF16, tag="rtmp",
                                           name="rtmp", bufs=2)
                            nc.vector.tensor_add(tmp[:], prb[:, 0, :],
                                                 prb[:, 1, :])
                            nc.vector.tensor_add(racc[:], racc[:], tmp[:])

                def sdpa_tail(h, qb):
                    qs = slice(qb * 512, (qb + 1) * 512)
                    # shares the "opj" rotation: 8-bank PSUM budget
                    rsps = ps3.tile([P, 512], F32, tag="opj", name="rs",
                                    bufs=2)
                    _mm(nc, rsps[:], ones_sb[:], racc_t.pop((h, qb))[:],
                        True, True)
                    rcp = rot.tile([P, 512], F32, tag="rcp", name="rcp",
                                   bufs=2)
                    nc.vector.reciprocal_approx_fast(rcp[:], rsps[:])
                    nc.vector.tensor_mul(y_sb[h][:, qs],
                                         yps_t.pop((h, qb))[:], rcp[:])

                def oproj(qb):
                    qs = slice(qb * 512, (qb + 1) * 512)
                    for oc4 in range(4):
                        osb = rot.tile([P, 4, 512], BF16, tag="osb",
                                       name="osb", bufs=2)
                        for j in range(4):
                            oc = 4 * oc4 + j
                            opt_ = ps3.tile([P, 512], F32, tag="opj",
                                            name="opj", bufs=2)
                            for hk in range(4):
                                _mm(nc, opt_[:],
                                    wo_sb[:, hk, oc * 128:(oc + 1) * 128],
                                    y_sb[hk][:, qs], hk == 0, hk == 3)
                            nc.vector.tensor_copy(osb[:, j, :], opt_[:])
                        dst = out_r[4 * oc4 * 128:(4 * oc4 + 4) * 128, qs]
                        nc.sync.dma_start(
                            dst.rearrange("(oc p) s -> p oc s", p=P), osb[:]
                        )

                # schedule: C00 up3 C10 T00 C20 T10 C30 T20 | C01 T30 OP0 ...
                prev = None          # core awaiting tail
                pending_op = None    # qb awaiting o_proj
                for qb in range(NQB):
                    for h in range(4):
                        sdpa_core(h, qb)
                        if qb == 0 and h == 0:
                            quarter_up(3, pool=ps3, ptag="opj")
                        if prev is not None:
                            sdpa_tail(*prev)
                        prev = (h, qb)
                        if pending_op is not None:
                            oproj(pending_op)
                            pending_op = None
                    pending_op = qb
                sdpa_tail(*prev)
                oproj(NQB - 1)

      if bench_io:
          with tc.tile_pool(name="bo", bufs=1) as bo:
              bt = bo.tile([1, 128], BF16, tag="bt", name="bt")
              nc.sync.dma_start(bt[:], out[0:1, 0:128])
              nc.sync.dma_start(outp[:, :], bt[:])

    nc.compile()
    return nc


_NC = None


def _get_nc():
    global _NC
    if _NC is None:
        _NC = build_program()
    return _NC


def _rope_tables():
    """cosA/sinB [128, S]: 32-row frequency pattern tiled 4x.
    sinB sign: rows [0:32] of each 64-block -> -sin, rows [32:64] -> +sin."""
    inv_freq = 1.0 / (BASE ** (np.arange(0, RD, 2, dtype=np.float32) / RD))  # [32]
    pos = np.arange(S, dtype=np.float32)
    ang = inv_freq[:, None] * pos[None, :]              # [32, S]
    cos1, sin1 = np.cos(ang), np.sin(ang)
    cosA = np.tile(cos1, (4, 1))                        # [128, S]
    sinB = np.concatenate([-sin1, sin1, -sin1, sin1], axis=0)
    return cosA.astype(_bf), sinB.astype(_bf)


def _mask_tiles():
    """Additive masks[k, d, q]: 0.0 if q >= d*128 + k else -300 (pre-scale;
    exp(scale*(s-300)-4) underflows to ~0)."""
    k = np.arange(P)[:, None]
    q = np.arange(512)[None, :]
    m = np.stack([np.where(q >= d * 128 + k, 0.0, -300.0)
                  for d in range(4)]).astype(np.float32)
    return np.ascontiguousarray(m.transpose(1, 0, 2)).astype(_bf)


def _prep_core_inputs(c, x, W_kv_d, W_q_d, W_k_u, W_q_u, W_v_u, W_rope_k, W_rope_q,
                      W_o, cosA, sinB, masks):
    b = c // 4
    hg = c % 4
    heads = [4 * hg + j for j in range(HEADS_PER_CORE)]

    def tile_pmaj(w):
        # [ko*128, m] -> [128, ko, m] partition-major for contiguous DMA
        ko = w.shape[0] // P
        return np.ascontiguousarray(
            w.reshape(ko, P, w.shape[1]).transpose(1, 0, 2))

    xT = np.ascontiguousarray(x[b].T).astype(_bf)                  # [H, S]
    w_kvd = tile_pmaj(np.ascontiguousarray(W_kv_d.T).astype(_bf))
    w_qd = tile_pmaj(np.ascontiguousarray(W_q_d.T).astype(_bf))

    # w_rk: per pair, rows [h1 rope dims | h0 rope dims], then transpose
    blocks = []
    for p in range(2):
        g0, g1 = heads[2 * p], heads[2 * p + 1]
        blocks.append(W_rope_k[g1 * RD:(g1 + 1) * RD, :])
        blocks.append(W_rope_k[g0 * RD:(g0 + 1) * RD, :])
    w_rk = tile_pmaj(np.ascontiguousarray(np.concatenate(blocks, axis=0).T).astype(_bf))

    # w_qc: per local head 128 cols: even -> [content|rope], odd -> [rope|content]
    cols = []
    for j, g in enumerate(heads):
        c_blk = W_q_u[g * RD:(g + 1) * RD, :].T       # [LAT, 64]
        r_blk = W_rope_q[g * RD:(g + 1) * RD, :].T    # [LAT, 64]
        cols.extend([c_blk, r_blk] if j % 2 == 0 else [r_blk, c_blk])
    w_qc = tile_pmaj(np.ascontiguousarray(np.concatenate(cols, axis=1)).astype(_bf))

    # w_kc: per pair 128 cols: [h0 content | h1 content]
    cols = []
    for p in range(2):
        g0, g1 = heads[2 * p], heads[2 * p + 1]
        cols.append(W_k_u[g0 * RD:(g0 + 1) * RD, :].T)
        cols.append(W_k_u[g1 * RD:(g1 + 1) * RD, :].T)
    w_kc = tile_pmaj(np.ascontiguousarray(np.concatenate(cols, axis=1)).astype(_bf))

    # w_v: per pair 256 cols: [h0 v dims | h1 v dims]
    cols = []
    for p in range(2):
        g0, g1 = heads[2 * p], heads[2 * p + 1]
        cols.append(W_v_u[g0 * HD:(g0 + 1) * HD, :].T)
        cols.append(W_v_u[g1 * HD:(g1 + 1) * HD, :].T)
    w_v = tile_pmaj(np.ascontiguousarray(np.concatenate(cols, axis=1)).astype(_bf))

    d0 = heads[0] * HD
    w_o = tile_pmaj(np.ascontiguousarray(W_o[:, d0:d0 + 512].T).astype(_bf))

    return {
        "xT": xT, "w_kvd": w_kvd, "w_qd": w_qd, "w_rk": w_rk, "w_qc": w_qc,
        "w_kc": w_kc, "w_v": w_v, "w_o": w_o, "cosA": cosA, "sinB": sinB,
        "masks": masks, "ident": np.eye(P, dtype=np.float32).astype(_bf),
    }


def make_in_maps(inputs):
    x = np.asarray(inputs["hidden_states"], dtype=np.float32)
    ws = {k: np.asarray(inputs[k], dtype=np.float32)
          for k in ("W_kv_d", "W_q_d", "W_k_u", "W_q_u", "W_v_u", "W_rope_k",
                    "W_rope_q", "W_o")}
    cosA, sinB = _rope_tables()
    masks = _mask_tiles()
    return [
        _prep_core_inputs(c, x, ws["W_kv_d"], ws["W_q_d"], ws["W_k_u"],
                          ws["W_q_u"], ws["W_v_u"], ws["W_rope_k"],
                          ws["W_rope_q"], ws["W_o"], cosA, sinB, masks)
        for c in range(N_CORES)
    ]


def assemble(results):
    """results: list of 8 dicts with 'out' [H, S] f32 partials (transposed)."""
    full = np.empty((B, S, H), dtype=np.float32)
    for b in range(B):
        acc = results[4 * b]["out"].astype(np.float32)
        for g in range(1, 4):
            acc = acc + results[4 * b + g]["out"]
        full[b] = acc.T
    return full


def kernel(**inputs):
    nc = _get_nc()
    in_maps = make_in_maps(inputs)
    res = run_bass_kernel_spmd(nc, in_maps, core_ids=list(range(N_CORES)))
    return assemble(res.results)


# revision 35
# speedup vs baseline: 1.1693x; 1.1693x over previous
"""MLA (multi-head latent attention) Trainium2 kernel.

Sharding: 8 cores = 2 (batch) x 4 (head groups of 4 heads).
Each core computes, for its batch b and heads [4g, 4g+4):
  latents kv_d/q_d (replicated within the batch group), per-head
  up-projections + RoPE, causal SDPA, and a partial o_proj
  out_core[o, q] = sum_{d in core's 512 head-dims} W_o[o, d] * y[d, q].
Host sums the 4 partials per batch (the all-reduce step of the hint,
performed at unshard time) and transposes to [S, H].

All matmuls run in bf16 with fp32 PSUM accumulation.

Phase 1 streams x once: per 512-col quarter, 6 matmuls per x k-tile
produce kv_d, q_d and raw rope-k together.  SDPA row-sums are
pre-reduced on DVE (bf16) with a single ones-matmul per (head,
q-block) for the partition reduction; softmax tails and o_proj are
deferred in program order so the Tensor engine never waits on them.
"""

import numpy as np
import ml_dtypes

import concourse.bass as bass
import concourse.mybir as mybir
import concourse.tile as tile
from concourse import bacc
from concourse._compat import get_trn_type
from concourse.bass_utils import run_bass_kernel_spmd

H = 2048
NH = 16
HD = 128           # head dim
RD = 64            # rotary dim
RH = 32            # rotary half
LAT = 256
B = 2
S = 2048
BASE = 10000.0
N_CORES = 8
HEADS_PER_CORE = 4
P = 128
NQB = S // 512     # 4 query blocks of 512
NKI = S // 128     # 16 key chunks of 128
SCALE = 1.0 / np.sqrt(float(HD))
EXP_BIAS = -4.0

BF16 = mybir.dt.bfloat16
F32 = mybir.dt.float32
_bf = ml_dtypes.bfloat16


def _mm(nc, out, lhsT, rhs, start, stop):
    nc.tensor.matmul(out, lhsT, rhs, start=start, stop=stop)


def build_program(nrep=1, bench_io=False):
    nc = bacc.Bacc(
        get_trn_type() or "TRN2",
        target_bir_lowering=False,
        debug=False,
        num_devices=N_CORES,
    )

    if bench_io:
        dummy = nc.declare_dram_parameter("bdummy", [1, 128], F32, isOutput=False)
        xT = nc.dram_tensor("xT", [H, S], BF16)
        w_kvd = nc.dram_tensor("w_kvd", [P, 16, LAT], BF16)
        w_qd = nc.dram_tensor("w_qd", [P, 16, LAT], BF16)
        w_rk = nc.dram_tensor("w_rk", [P, 16, 256], BF16)
        w_qc = nc.dram_tensor("w_qc", [P, 2, 512], BF16)
        w_kc = nc.dram_tensor("w_kc", [P, 2, 256], BF16)
        w_v = nc.dram_tensor("w_v", [P, 2, 512], BF16)
        w_o = nc.dram_tensor("w_o", [P, 4, H], BF16)
        cosA = nc.dram_tensor("cosA", [P, S], BF16)
        sinB = nc.dram_tensor("sinB", [P, S], BF16)
        masks = nc.dram_tensor("masks", [P, 4, 512], BF16)
        ident = nc.dram_tensor("ident", [P, P], BF16)
        out = nc.dram_tensor("outs", [H, S], BF16)
        outp = nc.declare_dram_parameter("out", [1, 128], BF16, isOutput=True)
    else:
        xT = nc.declare_dram_parameter("xT", [H, S], BF16, isOutput=False)
        w_kvd = nc.declare_dram_parameter("w_kvd", [P, 16, LAT], BF16, isOutput=False)
        w_qd = nc.declare_dram_parameter("w_qd", [P, 16, LAT], BF16, isOutput=False)
        w_rk = nc.declare_dram_parameter("w_rk", [P, 16, 256], BF16, isOutput=False)
        w_qc = nc.declare_dram_parameter("w_qc", [P, 2, 512], BF16, isOutput=False)
        w_kc = nc.declare_dram_parameter("w_kc", [P, 2, 256], BF16, isOutput=False)
        w_v = nc.declare_dram_parameter("w_v", [P, 2, 512], BF16, isOutput=False)
        w_o = nc.declare_dram_parameter("w_o", [P, 4, H], BF16, isOutput=False)
        cosA = nc.declare_dram_parameter("cosA", [P, S], BF16, isOutput=False)
        sinB = nc.declare_dram_parameter("sinB", [P, S], BF16, isOutput=False)
        masks = nc.declare_dram_parameter("masks", [P, 4, 512], BF16, isOutput=False)
        ident = nc.declare_dram_parameter("ident", [P, P], BF16, isOutput=False)
        out = nc.declare_dram_parameter("out", [H, S], BF16, isOutput=True)

    Exp = mybir.ActivationFunctionType.Exp

    scratch = (nc.dram_tensor("scratch", [H, S], BF16) if nrep > 1 else None)

    with tile.TileContext(nc) as tc:
      for rep in range(nrep):
        out_r = out if rep == nrep - 1 else scratch
        with (
            tc.tile_pool(name=f"wpool{rep}", bufs=1) as wpool,
            tc.tile_pool(name=f"main1_{rep}", bufs=1) as main1,
            tc.tile_pool(name=f"main2_{rep}", bufs=1) as main2,
            tc.tile_pool(name=f"ph1_{rep}", bufs=1) as ph1,
            tc.tile_pool(name=f"rot{rep}", bufs=3) as rot,
        ):
            # -------- persistent small tensors --------
            cos_sb = wpool.tile([P, S], BF16, tag="cos", name="cos")
            sin_sb = wpool.tile([P, S], BF16, tag="sin", name="sin")
            mask_sb = wpool.tile([P, 4, 512], BF16, tag="mask", name="mask")
            id_sb = wpool.tile([P, P], BF16, tag="ident", name="ident")
            ones_sb = wpool.tile([P, P], BF16, tag="ones", name="ones")
            nc.gpsimd.memset(ones_sb[:], 1.0)
            ebias_sb = wpool.tile([P, 1], F32, tag="ebias", name="ebias")
            nc.gpsimd.memset(ebias_sb[:], EXP_BIAS)
            wo_sb = wpool.tile([P, 4, H], BF16, tag="wo", name="wo")
            wqc_sb = wpool.tile([P, 2, 512], BF16, tag="wqc", name="wqc")
            wkc_sb = wpool.tile([P, 2, 256], BF16, tag="wkc", name="wkc")
            wv_sb = wpool.tile([P, 2, 512], BF16, tag="wv", name="wv")

            # -------- phase-1 outputs (latents + raw rope-k), bf16 --------
            kvd_sb = [main1.tile([P, S], BF16, tag=f"kvd{m}", name=f"kvd{m}")
                      for m in range(2)]
            qd_sb = [main1.tile([P, S], BF16, tag=f"qd{m}", name=f"qd{m}")
                     for m in range(2)]
            kraw = [main1.tile([P, S], BF16, tag=f"kraw{p}", name=f"kraw{p}")
                    for p in range(2)]

            # phase-1 weights: first k-chunks land before the x stream,
            # the bulk is split so arrival outpaces the k loop
            wkvd_sb = ph1.tile([P, 16, LAT], BF16, tag="wkvd", name="wkvd")
            wqd_sb = ph1.tile([P, 16, LAT], BF16, tag="wqd", name="wqd")
            wrk_sb = ph1.tile([P, 16, 256], BF16, tag="wrk", name="wrk")
            nc.sync.dma_start(wkvd_sb[:, 0:2, :], w_kvd[:, 0:2, :])
            nc.sync.dma_start(wqd_sb[:, 0:2, :], w_qd[:, 0:2, :])
            nc.sync.dma_start(wrk_sb[:, 0:2, :], w_rk[:, 0:2, :])

            # -------- per-head q/k tiles (dims on partitions), v, y --------
            # even head h: rows [0:64] content, [64:128] rope
            # odd  head h: rows [0:64] rope,    [64:128] content
            qT = [main2.tile([P, S], BF16, tag=f"qT{h}", name=f"qT{h}")
                  for h in range(4)]
            kT = [main2.tile([P, S], BF16, tag=f"kT{h}", name=f"kT{h}")
                  for h in range(4)]
            # v2[:, ki, p*256 + hv : +128] = v for head pair p, head h%2
            v2_sb = main2.tile([P, NKI, 512], BF16, tag="v2", name="v2")
            y_sb = [main2.tile([P, S], BF16, tag=f"y{h}", name=f"y{h}")
                    for h in range(4)]

            # -------- phases 1+2 fused: per 512-col quarter, the big
            # x-stream k-loop produces latents + raw rope-k; the
            # quarter's up-projections (k-content, q, v) run while the
            # NEXT quarter's k-loop keeps PE busy --------
            with tc.tile_pool(name=f"ps1_{rep}", bufs=1, space="PSUM") as ps1:

                def quarter_kloop(n4):
                    ns = slice(n4 * 512, (n4 + 1) * 512)
                    pkv = [ps1.tile([P, 512], F32, tag=f"pkv{m}",
                                    name=f"pkv{m}") for m in range(2)]
                    pqd = [ps1.tile([P, 512], F32, tag=f"pqd{m}",
                                    name=f"pqd{m}") for m in range(2)]
                    pkb = [ps1.tile([P, 512], F32, tag=f"pkb{p}",
                                    name=f"pkb{p}") for p in range(2)]
                    for k in range(16):
                        xtk = ph1.tile([P, 512], BF16, tag="xtk", name="xtk",
                                       bufs=8)
                        nc.sync.dma_start(
                            xtk[:],
                            xT[k * 128:(k + 1) * 128, ns],
                        )
                        if n4 == 0 and k in (1, 3, 5, 7, 9, 11, 13):
                            # bulk weights, arrival-paced 2-chunk pieces
                            ws = slice(k + 1, k + 3)
                            nc.sync.dma_start(wkvd_sb[:, ws, :],
                                              w_kvd[:, ws, :])
                            nc.sync.dma_start(wqd_sb[:, ws, :],
                                              w_qd[:, ws, :])
                            nc.sync.dma_start(wrk_sb[:, ws, :],
                                              w_rk[:, ws, :])
                        if n4 == 1 and k == 1:
                            nc.sync.dma_start(cos_sb[:], cosA[:, :])
                            nc.sync.dma_start(sin_sb[:], sinB[:, :])
                        if n4 == 1 and k == 3:
                            nc.sync.dma_start(wkc_sb[:], w_kc[:, :, :])
                            nc.sync.dma_start(wqc_sb[:], w_qc[:, :, :])
                        if n4 == 1 and k == 5:
                            nc.sync.dma_start(wv_sb[:], w_v[:, :, :])
                        if n4 == 2 and k == 2:
                            nc.sync.dma_start(id_sb[:], ident[:, :])
                            nc.sync.dma_start(mask_sb[:], masks[:, :, :])
                        if n4 == 2 and k == 8:
                            nc.sync.dma_start(wo_sb[:], w_o[:, :, :])
                        for m in range(2):
                            _mm(nc, pkv[m][:],
                                wkvd_sb[:, k, m * 128:(m + 1) * 128],
                                xtk[:], k == 0, k == 15)
                        for m in range(2):
                            _mm(nc, pqd[m][:],
                                wqd_sb[:, k, m * 128:(m + 1) * 128],
                                xtk[:], k == 0, k == 15)
                        for p in range(2):
                            _mm(nc, pkb[p][:],
                                wrk_sb[:, k, p * 128:(p + 1) * 128],
                                xtk[:], k == 0, k == 15)
                    return pkv, pqd, pkb

                def quarter_copies(n4, pkv, pqd, pkb):
                    ns = slice(n4 * 512, (n4 + 1) * 512)
                    # copies split across ACT/DVE
                    nc.scalar.copy(kvd_sb[0][:, ns], pkv[0][:])
                    nc.vector.tensor_copy(kvd_sb[1][:, ns], pkv[1][:])
                    nc.scalar.copy(qd_sb[0][:, ns], pqd[0][:])
                    nc.vector.tensor_copy(qd_sb[1][:, ns], pqd[1][:])
                    nc.scalar.copy(kraw[0][:, ns], pkb[0][:])
                    nc.vector.tensor_copy(kraw[1][:, ns], pkb[1][:])

                def quarter_krope(n4):
                    ns = slice(n4 * 512, (n4 + 1) * 512)
                    # rope on k for this quarter:
                    # kraw[p] rows [0:64]=h1 rope, [64:128]=h0 rope
                    for p in range(2):
                        h0, h1 = 2 * p, 2 * p + 1
                        swp = rot.tile([P, 512], BF16, tag="swp", name="swp",
                                       bufs=2)
                        nc.sync.dma_start(swp[0:32, :], kraw[p][32:64, ns])
                        nc.sync.dma_start(swp[32:64, :], kraw[p][0:32, ns])
                        nc.sync.dma_start(swp[64:96, :], kraw[p][96:128, ns])
                        nc.sync.dma_start(swp[96:128, :], kraw[p][64:96, ns])
                        nc.vector.tensor_mul(swp[:], swp[:], sin_sb[:, ns])
                        nc.vector.tensor_mul(
                            kT[h1][0:64, ns], kraw[p][0:64, ns],
                            cos_sb[0:64, ns]
                        )
                        nc.vector.tensor_add(
                            kT[h1][0:64, ns], kT[h1][0:64, ns], swp[0:64, :]
                        )
                        nc.vector.tensor_mul(
                            kT[h0][64:128, ns], kraw[p][64:128, ns],
                            cos_sb[64:128, ns]
                        )
                        nc.vector.tensor_add(
                            kT[h0][64:128, ns], kT[h0][64:128, ns],
                            swp[64:128, :]
                        )

                def quarter_up(n4, pool=None, ptag="ph2"):
                    # up-projections for quarter n4 (latents are ready)
                    pool = pool or ps1
                    ns = slice(n4 * 512, (n4 + 1) * 512)
                    # k content per pair: psum rows [0:64]=h0, [64:128]=h1
                    for p in range(2):
                        h0, h1 = 2 * p, 2 * p + 1
                        pt = pool.tile([P, 512], F32, tag=ptag, name="kcont",
                                       bufs=2)
                        for kc in range(2):
                            _mm(nc, pt[:],
                                wkc_sb[:, kc, p * 128:(p + 1) * 128],
                                kvd_sb[kc][:, ns], kc == 0, kc == 1)
                        nc.scalar.copy(kT[h0][0:64, ns], pt[0:64, :])
                        nc.vector.tensor_copy(kT[h1][64:128, ns],
                                              pt[64:128, :])
                    # q up-proj + rope per head
                    for h in range(4):
                        pt = pool.tile([P, 512], F32, tag=ptag, name="qcomb",
                                       bufs=2)
                        for kc in range(2):
                            _mm(nc, pt[:],
                                wqc_sb[:, kc, h * 128:(h + 1) * 128],
                                qd_sb[kc][:, ns], kc == 0, kc == 1)
                        nc.scalar.copy(qT[h][:, ns], pt[:])
                        r = 64 if h % 2 == 0 else 0
                        rs_ = slice(n4 * 512, (n4 + 1) * 512)
                        swp = rot.tile([P, 512], BF16, tag="swp", name="swpq",
                                       bufs=2)
                        nc.sync.dma_start(swp[r:r + 32, :],
                                          qT[h][r + 32:r + 64, rs_])
                        nc.sync.dma_start(swp[r + 32:r + 64, :],
                                          qT[h][r:r + 32, rs_])
                        nc.vector.tensor_mul(
                            qT[h][r:r + 64, ns], qT[h][r:r + 64, ns],
                            cos_sb[r:r + 64, ns]
                        )
                        nc.vector.tensor_mul(
                            swp[r:r + 64, :], swp[r:r + 64, :],
                            sin_sb[r:r + 64, ns]
                        )
                        nc.vector.tensor_add(
                            qT[h][r:r + 64, ns], qT[h][r:r + 64, ns],
                            swp[r:r + 64, :]
                        )
                    # v for this quarter's key chunks (both head pairs)
                    for i, s16 in enumerate(range(4 * n4, 4 * n4 + 4)):
                        pt = pool.tile([P, 512], F32, tag=ptag, name="vps",
                                       bufs=2)
                        for kc in range(2):
                            _mm(nc, pt[:],
                                kvd_sb[kc][:, s16 * 128:(s16 + 1) * 128],
                                wv_sb[:, kc, :], kc == 0, kc == 1)
                        if i % 2 == 0:
                            nc.vector.tensor_copy(v2_sb[:, s16, :], pt[:])
                        else:
                            nc.scalar.copy(v2_sb[:, s16, :], pt[:])

                # krope/up lag one quarter: they run while quarter n4+1's
                # k-loop owns the DMA stream, so their weights (issued in
                # quarter 1) never starve quarter 0's x tiles
                state = quarter_kloop(0)
                quarter_copies(0, *state)
                for n4 in range(1, 4):
                    state = quarter_kloop(n4)
                    quarter_krope(n4 - 1)
                    quarter_up(n4 - 1)
                    quarter_copies(n4, *state)
                quarter_krope(3)
                # quarter_up(3) is emitted inside the SDPA section after
                # the first head (which only needs quarter-0 products),
                # hiding its PSUM-copy chain under SDPA matmuls

            # -------- SDPA + o_proj --------
            # Emission schedule: per (h, qb) "core" = scores/exp/mask/pv +
            # DVE row-sum adds; "tail" = ones-matmul partition reduce +
            # rcp + y normalize, delayed one core-slot; o_proj(qb)
            # delayed past tail(3, qb) so PE never waits on DVE chains.
            with tc.tile_pool(name=f"ps3_{rep}", bufs=1, space="PSUM") as ps3:
                yps_t = {}
                racc_t = {}
                rsum_t = {}

                def sdpa_core(h, qb):
                    p = h // 2
                    hv = (h % 2) * 128
                    qs = slice(qb * 512, (qb + 1) * 512)
                    ngr = 2 * (qb + 1)    # groups of 2 k-chunks
                    # diagonal groups first so their extra mask step is
                    # off the tail
                    gs = [2 * qb, 2 * qb + 1] + list(range(2 * qb))
                    yps = ps3.tile([P, 512], F32, tag="ypv", name="ypv",
                                   bufs=2)
                    yps_t[(h, qb)] = yps
                    racc = rot.tile([P, 512], BF16, tag="racc", name="racc",
                                    bufs=2)
                    racc_t[(h, qb)] = racc
                    for gi, g in enumerate(gs):
                        diag = g // 2 == qb
                        scps = ps3.tile([P, 2, 512], F32, tag="sc",
                                        name="sc", bufs=2)
                        for j in range(2):
                            ki = 2 * g + j
                            _mm(nc, scps[:, j, :],
                                kT[h][:, ki * 128:(ki + 1) * 128],
                                qT[h][:, qs], True, not diag)
                            if diag:
                                # additive causal mask folded into PSUM
                                _mm(nc, scps[:, j, :], id_sb[:],
                                    mask_sb[:, 2 * (g % 2) + j, :],
                                    False, True)
                        prb = rot.tile([P, 2, 512], BF16, tag="prb",
                                       name="prb")
                        nc.scalar.activation(
                            prb[:], scps[:], Exp, bias=ebias_sb[:],
                            scale=SCALE
                        )
                        for j in range(2):
                            _mm(nc, yps[:],
                                v2_sb[:, 2 * g + j, p * 256 + hv:
                                      p * 256 + hv + 128],
                                prb[:, j, :], gi == 0 and j == 0,
                                gi == ngr - 1 and j == 1)
                        # row-sum pre-reduction on DVE (bf16, 2x rate)
                        if gi == 0:
                            nc.vector.tensor_add(racc[:], prb[:, 0, :],
                                                 prb[:, 1, :])
                        else:
                            tmp = rot.tile([P, 512], BF16, tag="rtmp",
                                           name="rtmp", bufs=2)
                            nc.vector.tensor_add(tmp[:], prb[:, 0, :],
                                                 prb[:, 1, :])
                            nc.vector.tensor_add(racc[:], racc[:], tmp[:])

                def sdpa_tail(h, qb):
                    qs = slice(qb * 512, (qb + 1) * 512)
                    # shares the "opj" rotation: 8-bank PSUM budget
                    rsps = ps3.tile([P, 512], F32, tag="opj", name="rs",
                                    bufs=2)
                    _mm(nc, rsps[:], ones_sb[:], racc_t.pop((h, qb))[:],
                        True, True)
                    rcp = rot.tile([P, 512], F32, tag="rcp", name="rcp",
                                   bufs=2)
                    nc.vector.reciprocal_approx_fast(rcp[:], rsps[:])
                    nc.vector.tensor_mul(y_sb[h][:, qs],
                                         yps_t.pop((h, qb))[:], rcp[:])

                def oproj(qb):
                    qs = slice(qb * 512, (qb + 1) * 512)
                    for oc4 in range(4):
                        osb = rot.tile([P, 4, 512], BF16, tag="osb",
                                       name="osb", bufs=2)
                        for j in range(4):
                            oc = 4 * oc4 + j
                            opt_ = ps3.tile([P, 512], F32, tag="opj",
                                            name="opj", bufs=2)
                            for hk in range(4):
                                _mm(nc, opt_[:],
                                    wo_sb[:, hk, oc * 128:(oc + 1) * 128],
                                    y_sb[hk][:, qs], hk == 0, hk == 3)
                            nc.vector.tensor_copy(osb[:, j, :], opt_[:])
                        dst = out_r[4 * oc4 * 128:(4 * oc4 + 4) * 128, qs]
                        nc.sync.dma_start(
                            dst.rearrange("(oc p) s -> p oc s", p=P), osb[:]
                        )

                # schedule: C00 up3 C10 T00 C20 T10 C30 T20 | C01 T30 OP0 ...
                prev = None          # core awaiting tail
                pending_op = None    # qb awaiting o_proj
                for qb in range(NQB):
                    for h in range(4):
                        sdpa_core(h, qb)
                        if qb == 0 and h == 0:
                            quarter_up(3, pool=ps3, ptag="opj")
                        if prev is not None:
                            sdpa_tail(*prev)
                        prev = (h, qb)
                        if pending_op is not None:
                            oproj(pending_op)
                            pending_op = None
                    pending_op = qb
                sdpa_tail(*prev)
                oproj(NQB - 1)

      if bench_io:
          with tc.tile_pool(name="bo", bufs=1) as bo:
              bt = bo.tile([1, 128], BF16, tag="bt", name="bt")
              nc.sync.dma_start(bt[:], out[0:1, 0:128])
              nc.sync.dma_start(outp[:, :], bt[:])

    nc.compile()
    return nc


_NC = None


def _get_nc():
    global _NC
    if _NC is None:
        _NC = build_program()
    return _NC


def _rope_tables():
    """cosA/sinB [128, S]: 32-row frequency pattern tiled 4x.
    sinB sign: rows [0:32] of each 64-block -> -sin, rows [32:64] -> +sin."""
    inv_freq = 1.0 / (BASE ** (np.arange(0, RD, 2, dtype=np.float32) / RD))  # [32]
    pos = np.arange(S, dtype=np.float32)
    ang = inv_freq[:, None] * pos[None, :]              # [32, S]
    cos1, sin1 = np.cos(ang), np.sin(ang)
    cosA = np.tile(cos1, (4, 1))                        # [128, S]
    sinB = np.concatenate([-sin1, sin1, -sin1, sin1], axis=0)
    return cosA.astype(_bf), sinB.astype(_bf)


def _mask_tiles():
    """Additive masks[k, d, q]: 0.0 if q >= d*128 + k else -300 (pre-scale;
    exp(scale*(s-300)-4) underflows to ~0)."""
    k = np.arange(P)[:, None]
    q = np.arange(512)[None, :]
    m = np.stack([np.where(q >= d * 128 + k, 0.0, -300.0)
                  for d in range(4)]).astype(np.float32)
    return np.ascontiguousarray(m.transpose(1, 0, 2)).astype(_bf)


def _prep_core_inputs(c, x, W_kv_d, W_q_d, W_k_u, W_q_u, W_v_u, W_rope_k, W_rope_q,
                      W_o, cosA, sinB, masks):
    b = c // 4
    hg = c % 4
    heads = [4 * hg + j for j in range(HEADS_PER_CORE)]

    def tile_pmaj(w):
        # [ko*128, m] -> [128, ko, m] partition-major for contiguous DMA
        ko = w.shape[0] // P
        return np.ascontiguousarray(
            w.reshape(ko, P, w.shape[1]).transpose(1, 0, 2))

    xT = np.ascontiguousarray(x[b].T).astype(_bf)                  # [H, S]
    w_kvd = tile_pmaj(np.ascontiguousarray(W_kv_d.T).astype(_bf))
    w_qd = tile_pmaj(np.ascontiguousarray(W_q_d.T).astype(_bf))

    # w_rk: per pair, rows [h1 rope dims | h0 rope dims], then transpose
    blocks = []
    for p in range(2):
        g0, g1 = heads[2 * p], heads[2 * p + 1]
        blocks.append(W_rope_k[g1 * RD:(g1 + 1) * RD, :])
        blocks.append(W_rope_k[g0 * RD:(g0 + 1) * RD, :])
    w_rk = tile_pmaj(np.ascontiguousarray(np.concatenate(blocks, axis=0).T).astype(_bf))

    # w_qc: per local head 128 cols: even -> [content|rope], odd -> [rope|content]
    cols = []
    for j, g in enumerate(heads):
        c_blk = W_q_u[g * RD:(g + 1) * RD, :].T       # [LAT, 64]
        r_blk = W_rope_q[g * RD:(g + 1) * RD, :].T    # [LAT, 64]
        cols.extend([c_blk, r_blk] if j % 2 == 0 else [r_blk, c_blk])
    w_qc = tile_pmaj(np.ascontiguousarray(np.concatenate(cols, axis=1)).astype(_bf))

    # w_kc: per pair 128 cols: [h0 content | h1 content]
    cols = []
    for p in range(2):
        g0, g1 = heads[2 * p], heads[2 * p + 1]
        cols.append(W_k_u[g0 * RD:(g0 + 1) * RD, :].T)
        cols.append(W_k_u[g1 * RD:(g1 + 1) * RD, :].T)
    w_kc = tile_pmaj(np.ascontiguousarray(np.concatenate(cols, axis=1)).astype(_bf))

    # w_v: per pair 256 cols: [h0 v dims | h1 v dims]
    cols = []
    for p in range(2):
        g0, g1 = heads[2 * p], heads[2 * p + 1]
        cols.append(W_v_u[g0 * HD:(g0 + 1) * HD, :].T)
        cols.append(W_v_u[g1 * HD:(g1 + 1) * HD, :].T)
    w_v = tile_pmaj(np.ascontiguousarray(np.concatenate(cols, axis=1)).astype(_bf))

    d0 = heads[0] * HD
    w_o = tile_pmaj(np.ascontiguousarray(W_o[:, d0:d0 + 512].T).astype(_bf))

    return {
        "xT": xT, "w_kvd": w_kvd, "w_qd": w_qd, "w_rk": w_rk, "w_qc": w_qc,
        "w_kc": w_kc, "w_v": w_v, "w_o": w_o, "cosA": cosA, "sinB": sinB,
        "masks": masks, "ident": np.eye(P, dtype=np.float32).astype(_bf),
    }


def make_in_maps(inputs):
    x = np.asarray(inputs["hidden_states"], dtype=np.float32)
    ws = {k: np.asarray(inputs[k], dtype=np.float32)
          for k in ("W_kv_d", "W_q_d", "W_k_u", "W_q_u", "W_v_u", "W_rope_k",
                    "W_rope_q", "W_o")}
    cosA, sinB = _rope_tables()
    masks = _mask_tiles()
    return [
        _prep_core_inputs(c, x, ws["W_kv_d"], ws["W_q_d"], ws["W_k_u"],
                          ws["W_q_u"], ws["W_v_u"], ws["W_rope_k"],
                          ws["W_rope_q"], ws["W_o"], cosA, sinB, masks)
        for c in range(N_CORES)
    ]


def assemble(results):
    """results: list of 8 dicts with 'out' [H, S] f32 partials (transposed)."""
    full = np.empty((B, S, H), dtype=np.float32)
    for b in range(B):
        acc = results[4 * b]["out"].astype(np.float32)
        for g in range(1, 4):
            acc = acc + results[4 * b + g]["out"]
        full[b] = acc.T
    return full


def kernel(**inputs):
    nc = _get_nc()
    in_maps = make_in_maps(inputs)
    res = run_bass_kernel_spmd(nc, in_maps, core_ids=list(range(N_CORES)))
    return assemble(res.results)
